# revision 16
# baseline (speedup 1.0000x reference)
"""GCN (2-layer, PyG-style gcn_norm) Bass/Tile kernel for Trainium2, 8 NeuronCores.

Strategy (dst-partitioned message passing, dense-packed gathers, separable norm):
  - Nodes are partitioned across 8 cores by destination; every edge is routed
    to the core that owns its destination node. Self-loop + symmetric
    D^-1/2 A D^-1/2 normalization is computed on the host (index/routing
    preprocessing only).
  - The norm dinv[src]*dinv[dst] is SEPARABLE: dinv[src] is folded into the
    gather tables (x is pre-scaled on the host; the layer-1 epilogue scales
    the h@W2 table rows), and dinv[dst] is applied once per output tile.
    The per-edge selection matrices are then PURE 0/1 one-hots, built by one
    single-op DVE tensor_scalar per block: S[e, c] = (iota == dl_e), where
    out-of-tile / padding slots carry the sentinel dl = 300 (all-zero row).
  - The SWDGE gather drain is byte-bound, so each layer picks the smallest
    legal descriptor: layer 1 gathers single 256B rows of dinv*x
    ([100000, 128] bf16, 4 int16-reach buckets); layer 2 gathers 256B PAIR
    rows of dinv*(h@W2) ([50176, 128] bf16, 2 buckets) with the parity
    selecting the half (dl' = dst_local + 128*parity, two matmuls per block
    on the S_cat halves).
  - Messages are packed DENSELY: per (bucket, dst-tile) group sized to
    roundup16(max-over-cores count) so all 8 cores run one program; 128-slot
    blocks cut across group boundaries, and a block shared by adjacent tiles
    is matmul'd once per tile with the other tile's slots sentinel-masked.
  - Matmuls accumulate p1[node, feat] += S^T @ msg in PSUM (nodes on
    partitions). Self-loops never enter the gather stream: each tile's own
    rows arrive by cheap sequential HWDGE DMA (from a per-core xown input /
    the local h bounce buffer) and are scattered with constant selection
    matrices (identity for layer 1, a fixed [64, 256] pattern for layer 2).
  - Layer-1 epilogue: dinv[dst] scale, transpose, W1 matmul, bias+ReLU
    (ScalarE), W2 matmul folded in BEFORE the halo exchange (the AllGather
    ships h@W2, 64 wide -- half the bytes), transpose, dinv scale.
  - Layer-2 bias enters as a K=1 rank-1 matmul (b2 x sqrt(deg)) inside the
    PSUM accumulation; the [node, feat] output orientation is already
    row-major, so layer 2 finishes with just scale, ReLU, store.
"""

import os
from dataclasses import dataclass

import numpy as np

P = 128
PAD_DL = 300.0  # sentinel: one-hot of 300 over iota 0..255 is all-zero


@dataclass(frozen=True)
class Geom:
    n_nodes: int
    n_cores: int
    in_dim: int
    h1: int
    h2: int
    gcols1: int  # layer-1 gather-group size, in 128-slot blocks per call
    gcols2: int  # layer-2 gather-group size
    selfk: int  # tiles per sequential self-row DMA chunk
    mm_bf16: bool  # bf16 tables/matmul operands (accumulation stays f32)

    @property
    def shard(self) -> int:
        return -(-self.n_nodes // self.n_cores)

    @property
    def tiles(self) -> int:
        return -(-self.shard // P)

    @property
    def shard_pad(self) -> int:
        return self.tiles * P


def _pack_layer(core, t_idx, dl_vals, row, nbuck, bsz, tiles, n_cores, gcols):
    """Dense pad-16 packing of one layer's messages.

    Returns dict with per-core idx [P, nb*8] (wrap16), dl [P, n_mm] (f32,
    PAD_DL sentinels), plus layout: nb (blocks), calls, sched (per tile:
    list of (block, mm_col)).
    """
    buck = row // bsz
    val = (row - buck * bsz).astype(np.int16)
    assert int(row.max()) - int(buck.max()) * bsz < 32768

    gkey = (core * nbuck + buck) * tiles + t_idx
    ngrp = n_cores * nbuck * tiles
    cnt = np.bincount(gkey, minlength=ngrp).reshape(n_cores, nbuck, tiles)
    size_bt = ((cnt.max(axis=0) + 15) // 16) * 16  # [nbuck, tiles]

    # group slot starts, bucket-major; bucket totals padded to whole blocks
    start_bt = np.zeros((nbuck, tiles), dtype=np.int64)
    off = 0
    bucket_span = []  # (block0, nblocks) per bucket
    for b in range(nbuck):
        blk0 = off // P
        for t in range(tiles):
            start_bt[b, t] = off
            off += int(size_bt[b, t])
        off = ((off + P - 1) // P) * P
        bucket_span.append((blk0, off // P - blk0))
    nb = off // P

    # matmul schedule: per (b, t) the overlapped blocks, t-major mm columns
    sched = [[] for _ in range(tiles)]
    k0_bt = np.zeros((nbuck, tiles), dtype=np.int64)
    jstart_bt = np.zeros((nbuck, tiles), dtype=np.int64)
    j = 0
    for t in range(tiles):
        for b in range(nbuck):
            s, e = int(start_bt[b, t]), int(start_bt[b, t] + size_bt[b, t])
            if e == s:
                continue
            ks = range(s // P, (e + P - 1) // P)
            k0_bt[b, t] = s // P
            jstart_bt[b, t] = j
            for k in ks:
                sched[t].append((k, j))
                j += 1
    n_mm = j

    # place each message: global slot, block, lane, mm column
    order = np.argsort(gkey, kind="stable")
    gs = np.zeros(ngrp + 1, dtype=np.int64)
    np.cumsum(np.bincount(gkey, minlength=ngrp), out=gs[1:])
    pos = np.arange(len(gkey), dtype=np.int64) - gs[gkey[order]]

    ci = core[order]
    b_o, t_o = buck[order], t_idx[order]
    slot = start_bt[b_o, t_o] + pos
    kblk = slot // P
    lane = slot % P
    jcol = jstart_bt[b_o, t_o] + (kblk - k0_bt[b_o, t_o])

    idxm = np.zeros((n_cores, P, nb), dtype=np.int16)
    dlm = np.full((n_cores, P, n_mm), PAD_DL, dtype=np.float32)
    idxm[ci, lane, kblk] = val[order]
    dlm[ci, lane, jcol] = dl_vals[order]

    calls = []
    for blk0, nblk in bucket_span:
        c0 = blk0
        while c0 < blk0 + nblk:
            k = min(gcols, blk0 + nblk - c0)
            calls.append((c0, k, len(calls)))
            c0 += k
    # rewrite third field as bucket id for table slicing
    calls = [
        (c0, k, next(b for b, (b0, nn) in enumerate(bucket_span) if b0 <= c0 < b0 + nn))
        for (c0, k, _x) in calls
    ]

    def wrap16(mat):
        out = np.zeros((P, nb * 8), dtype=np.int16)
        for c0, k, _b in calls:
            seg = mat[:, c0 : c0 + k].T.reshape(-1)
            out[:16, c0 * 8 : (c0 + k) * 8] = seg.reshape(k * 8, 16).T
        return np.tile(out[:16], (8, 1))

    per_core = [dict(idx=wrap16(idxm[i]), dl=dlm[i]) for i in range(n_cores)]
    layout = dict(nb=nb, n_mm=n_mm, calls=calls, sched=sched)
    return per_core, layout


def preprocess(edge_index: np.ndarray, g: Geom):
    n, c, shard, tiles = g.n_nodes, g.n_cores, g.shard, g.tiles
    src = edge_index[0].astype(np.int64)
    dst = edge_index[1].astype(np.int64)

    deg = np.bincount(dst, minlength=n).astype(np.float32) + 1.0  # + self loop
    dinv = (1.0 / np.sqrt(deg)).astype(np.float32)

    core = dst // shard
    local = dst - core * shard
    t_idx = local // P
    dl = (local % P).astype(np.float32)

    # layer 1: single rows of x, 4 buckets
    pc1, lay1 = _pack_layer(
        core, t_idx, dl, src, 4, -(-n // 4), tiles, c, g.gcols1
    )
    # layer 2: pair rows of h@W2, 2 buckets; parity in dl'
    src2 = (src // shard) * g.shard_pad + (src % shard)
    npair2 = c * g.shard_pad // 2
    dlp = dl + P * (src2 % 2).astype(np.float32)
    pc2, lay2 = _pack_layer(
        core, t_idx, dlp, src2 // 2, 2, -(-npair2 // 2), tiles, c, g.gcols2
    )

    dpad = np.zeros(c * g.shard_pad, dtype=np.float32)
    spad = np.ones(c * g.shard_pad, dtype=np.float32)
    for i in range(c):
        lo, hi = i * shard, (i + 1) * shard
        dpad[i * g.shard_pad : i * g.shard_pad + shard] = dinv[lo:hi]
        spad[i * g.shard_pad : i * g.shard_pad + shard] = np.sqrt(deg[lo:hi])

    per_core = [
        dict(
            gidx1=pc1[i]["idx"],
            gidx2=pc2[i]["idx"],
            dl1=pc1[i]["dl"],
            dl2=pc2[i]["dl"],
            ddst=dpad[i * g.shard_pad : (i + 1) * g.shard_pad]
            .reshape(tiles, P)
            .T.copy(),
            sdeg=spad[i * g.shard_pad : (i + 1) * g.shard_pad][None, :].copy(),
        )
        for i in range(c)
    ]
    return per_core, dict(l1=lay1, l2=lay2), dinv


def build_program(g: Geom, layout):
    import concourse.bass as bass  # noqa: F401
    import concourse.mybir as mybir
    import concourse.tile as tile
    from concourse import bacc, library_config

    f32 = mybir.dt.float32
    i16 = mybir.dt.int16
    mm_dt = mybir.dt.bfloat16 if g.mm_bf16 else mybir.dt.float32

    shard, tiles, shard_pad = g.shard, g.tiles, g.shard_pad
    ablate = set(os.environ.get("GCN_ABLATE", "").split(","))  # timing experiments
    stage = os.environ.get("GCN_STAGE", "full")  # g | gs | gsm | full
    nq = int(os.environ.get("GCN_NQ", "4"))
    sp = os.environ.get("GCN_SP", "1") == "1"

    lay1, lay2 = layout["l1"], layout["l2"]
    nb1, nb2 = lay1["nb"], lay2["nb"]
    nmm1, nmm2 = lay1["n_mm"], lay2["n_mm"]
    npair2 = g.n_cores * shard_pad // 2
    elem1 = g.in_dim  # single-row width (256B bf16)
    elem2 = 2 * g.h2  # pair-row width, 128 elements (256B bf16)
    bsz1 = -(-g.n_nodes // 4)
    bsz2 = -(-npair2 // 2)
    mt_free = g.gcols1 * elem1
    assert g.gcols2 * elem2 <= mt_free
    selfk = g.selfk
    nchunk = -(-tiles // selfk)

    nc = bacc.Bacc(
        "TRN2",
        target_bir_lowering=False,
        debug=False,
        enable_asserts=False,
        num_devices=g.n_cores,
        num_swdge_queues=nq,
        dynamic_dma_scratch_size=int(os.environ.get("GCN_SCRATCH", "65536")),
    )

    x_d = nc.dram_tensor("x", [g.n_nodes, elem1], mm_dt, kind="ExternalInput")
    xo_d = nc.dram_tensor("xown", [shard_pad, elem1], mm_dt, kind="ExternalInput")
    gi1_d = nc.dram_tensor("gidx1", [P, nb1 * 8], i16, kind="ExternalInput")
    gi2_d = nc.dram_tensor("gidx2", [P, nb2 * 8], i16, kind="ExternalInput")
    dl1_d = nc.dram_tensor("dl1", [P, nmm1], f32, kind="ExternalInput")
    dl2_d = nc.dram_tensor("dl2", [P, nmm2], f32, kind="ExternalInput")
    dd_d = nc.dram_tensor("ddst", [P, tiles], f32, kind="ExternalInput")
    sd_d = nc.dram_tensor("sdeg", [1, tiles * P], f32, kind="ExternalInput")
    w1_d = nc.dram_tensor("w1", [g.in_dim, g.h1], mm_dt, kind="ExternalInput")
    w2_d = nc.dram_tensor("w2", [g.h1, g.h2], mm_dt, kind="ExternalInput")
    b1_d = nc.dram_tensor("b1", [g.h1], f32, kind="ExternalInput")
    b2r_d = nc.dram_tensor("b2row", [1, g.h2], f32, kind="ExternalInput")
    io_d = nc.dram_tensor("iotam", [P, 2 * P], mm_dt, kind="ExternalInput")
    ss_d = nc.dram_tensor("sself", [P // 2, 2 * P], mm_dt, kind="ExternalInput")
    idm_d = nc.dram_tensor("identm", [P, P], mm_dt, kind="ExternalInput")
    out_d = nc.dram_tensor("out", [shard, g.h2], f32, kind="ExternalOutput")

    hb_d = nc.dram_tensor("h_bounce", [shard_pad, g.h2], mm_dt, kind="Internal")
    hf_d = nc.dram_tensor(
        "h_full", [npair2, elem2], mm_dt, kind="Internal", addr_space="Shared"
    )

    with tile.TileContext(nc) as tc:
        with (
            tc.tile_pool(name="const", bufs=1) as cpool,
            tc.tile_pool(name="msg", bufs=int(os.environ.get("GCN_MBUFS", "14"))) as mpool,
            tc.tile_pool(name="selfp", bufs=3) as fpool,
            tc.tile_pool(name="sel", bufs=int(os.environ.get("GCN_SBUFS", "12"))) as spool,
            tc.tile_pool(name="act", bufs=4) as apool,
            tc.tile_pool(name="psum", bufs=2, space="PSUM") as ppool,
        ):
            nc.gpsimd.load_library(library_config.mlp)

            iota2 = cpool.tile([P, 2 * P], mm_dt, tag="iota2")
            nc.sync.dma_start(out=iota2[:], in_=io_d[:, :])
            sself = cpool.tile([P // 2, 2 * P], mm_dt, tag="sself")
            nc.sync.dma_start(out=sself[:], in_=ss_d[:, :])
            ident = cpool.tile([P, P], mm_dt, tag="ident")
            nc.sync.dma_start(out=ident[:], in_=idm_d[:, :])

            w1_s = cpool.tile([g.in_dim, g.h1], mm_dt, tag="w1")
            nc.sync.dma_start(out=w1_s[:], in_=w1_d[:, :])
            w2_s = cpool.tile([g.h1, g.h2], mm_dt, tag="w2")
            nc.sync.dma_start(out=w2_s[:], in_=w2_d[:, :])
            b1_s = cpool.tile([g.h1, 1], f32, tag="b1")
            nc.sync.dma_start(out=b1_s[:], in_=b1_d[:, None])
            b2r_s = cpool.tile([1, g.h2], f32, tag="b2r")
            nc.sync.dma_start(out=b2r_s[:], in_=b2r_d[:, :])
            dd_s = cpool.tile([P, tiles], f32, tag="ddst")
            nc.sync.dma_start(out=dd_s[:], in_=dd_d[:, :])
            sd_s = cpool.tile([1, tiles * P], f32, tag="sdeg")
            nc.sync.dma_start(out=sd_s[:], in_=sd_d[:, :])

            gidx_s = cpool.tile([P, max(nb1, nb2) * 8], i16, tag="gidx")
            dl_s = cpool.tile([P, max(nmm1, nmm2)], f32, tag="dl")

            def layer(gi_dram, dl_dram, nb, lay, table_bucket_ap, self_chunk_ap,
                      elem, fh, pair, rank1_bias, epilogue):
                calls = lay["calls"]
                sched = lay["sched"]
                col2call = np.zeros(nb, dtype=np.int64)
                for ci_, (c0, k, _b) in enumerate(calls):
                    col2call[c0 : c0 + k] = ci_
                nc.sync.dma_start(out=gidx_s[:, : nb * 8], in_=gi_dram[:, :])
                nc.sync.dma_start(out=dl_s[:, : lay["n_mm"]], in_=dl_dram[:, :])
                msg_tiles: dict[int, object] = {}
                self_tiles: dict[int, object] = {}

                def ensure_call(ci_: int):
                    if ci_ in msg_tiles:
                        return
                    c0, k, b = calls[ci_]
                    mt = mpool.tile([P, mt_free], mm_dt, tag="msg")
                    if "gather" in ablate:
                        msg_tiles[ci_] = mt
                        return
                    nc.gpsimd.dma_gather(
                        queue_num=ci_ % nq,
                        out_ap=mt[:].rearrange("p (k d) -> p k d", d=elem)[:, :k, :],
                        in_ap=table_bucket_ap(b),
                        idxs_ap=gidx_s[:, c0 * 8 : (c0 + k) * 8],
                        num_idxs=k * P,
                        num_idxs_reg=k * P,
                        elem_size=elem,
                        single_packet=sp,
                    )
                    msg_tiles[ci_] = mt

                def ensure_self(ch: int):
                    if ch in self_tiles:
                        return
                    t0 = ch * selfk
                    kk = min(selfk, tiles - t0)
                    prt = P if not pair else P // 2
                    ft = fpool.tile([P, selfk * elem1], mm_dt, tag="selfmt")
                    nc.sync.dma_start(
                        out=ft[:prt, : kk * elem].rearrange(
                            "i (t e) -> i t e", e=elem
                        ),
                        in_=self_chunk_ap(t0, kk),
                    )
                    self_tiles[ch] = ft

                for t in range(tiles):
                    ensure_self(t // selfk)
                    if t // selfk + 1 < nchunk and t % selfk >= selfk - 2:
                        ensure_self(t // selfk + 1)
                    if stage in ("gsm", "full"):
                        p1 = ppool.tile([P, P], f32, tag="p1", space="PSUM")
                        if rank1_bias:
                            nc.tensor.matmul(
                                p1[:, :fh],
                                lhsT=sd_s[:, t * P : (t + 1) * P],
                                rhs=b2r_s[:, :],
                                start=True,
                                stop=False,
                            )
                        ft = self_tiles[t // selfk]
                        so = (t % selfk) * elem
                        if pair:
                            nc.tensor.matmul(
                                p1[:, :fh],
                                lhsT=sself[:, 0:P],
                                rhs=ft[: P // 2, so : so + fh],
                                start=not rank1_bias,
                                stop=False,
                            )
                            nc.tensor.matmul(
                                p1[:, :fh],
                                lhsT=sself[:, P : 2 * P],
                                rhs=ft[: P // 2, so + fh : so + 2 * fh],
                                start=False,
                                stop=False,
                            )
                        else:
                            nc.tensor.matmul(
                                p1[:, :fh],
                                lhsT=ident[:, :],
                                rhs=ft[:, so : so + fh],
                                start=not rank1_bias,
                                stop=False,
                            )
                    for i_, (k, jmm) in enumerate(sched[t]):
                        ci_ = int(col2call[k])
                        ensure_call(ci_)
                        if ci_ + 1 < len(calls) and k - calls[ci_][0] >= calls[ci_][1] - 3:
                            ensure_call(ci_ + 1)
                        if stage == "g":
                            continue
                        off = k - calls[ci_][0]
                        wid = 2 * P if pair else P
                        s_t = spool.tile([P, 2 * P], mm_dt, tag="S")
                        nc.vector.tensor_scalar(
                            s_t[:, :wid],
                            iota2[:, :wid],
                            dl_s[:, jmm : jmm + 1],
                            None,
                            op0=mybir.AluOpType.is_equal,
                        )
                        if stage == "gs":
                            continue
                        mt = msg_tiles[ci_]
                        last = i_ == len(sched[t]) - 1
                        nc.tensor.matmul(
                            p1[:, :fh],
                            lhsT=s_t[:, 0:P],
                            rhs=mt[:, off * elem : off * elem + fh],
                            start=False,
                            stop=last and not pair,
                        )
                        if pair:
                            nc.tensor.matmul(
                                p1[:, :fh],
                                lhsT=s_t[:, P : 2 * P],
                                rhs=mt[:, off * elem + fh : off * elem + 2 * fh],
                                start=False,
                                stop=last,
                            )
                    if stage in ("g", "gs", "gsm"):
                        continue
                    epilogue(t, p1)

            def epi_l1(t, p1):
                # p1[node, in_dim] aggregated; scale by dinv[dst], cast bf16
                a1 = apool.tile([P, P], mm_dt, tag="a1")
                nc.vector.tensor_scalar(
                    a1[:, :], p1[:, :], dd_s[:, t : t + 1], None,
                    op0=mybir.AluOpType.mult,
                )
                ptr = ppool.tile([P, P], mm_dt, tag="ptr", space="PSUM")
                nc.tensor.transpose(ptr[:, :], a1[:, :], ident[:, :])
                a2 = apool.tile([P, P], mm_dt, tag="a2")
                nc.vector.tensor_copy(a2[:, :], ptr[:, :])
                p2 = ppool.tile([P, P], f32, tag="p2", space="PSUM")
                nc.tensor.matmul(
                    p2[:, :], lhsT=w1_s[:, :], rhs=a2[:, :], start=True, stop=True
                )
                ht = apool.tile([P, P], mm_dt, tag="ht")
                nc.scalar.activation(
                    ht[:, :], p2[:, :],
                    mybir.ActivationFunctionType.Relu,
                    bias=b1_s[:, :],
                )
                p2b = ppool.tile([P, P], f32, tag="p2b", space="PSUM")
                nc.tensor.matmul(
                    p2b[: g.h2, :], lhsT=w2_s[:, : g.h2], rhs=ht[:, :],
                    start=True, stop=True,
                )
                h2t = apool.tile([P, P], mm_dt, tag="h2t")
                nc.vector.tensor_copy(h2t[: g.h2, :], p2b[: g.h2, :])
                pt = ppool.tile([P, P], mm_dt, tag="ptr", space="PSUM")
                nc.tensor.transpose(pt[:, : g.h2], h2t[: g.h2, :], ident[: g.h2, : g.h2])
                hrow = apool.tile([P, P], mm_dt, tag="hrow")
                nc.vector.tensor_scalar(
                    hrow[:, : g.h2], pt[:, : g.h2], dd_s[:, t : t + 1], None,
                    op0=mybir.AluOpType.mult,
                )
                nc.sync.dma_start(out=hb_d[t * P : (t + 1) * P, :], in_=hrow[:, : g.h2])

            def epi_l2(t, p1):
                # p1[node, h2] aggregated (incl. rank-1 bias term);
                # out = relu(dinv[dst] * p1) -- already row-major
                hrow = apool.tile([P, P], f32, tag="hrowf")
                nc.scalar.activation(
                    hrow[:, : g.h2], p1[:, : g.h2],
                    mybir.ActivationFunctionType.Relu,
                    scale=dd_s[:, t : t + 1],
                )
                rows = min(P, shard - t * P)
                nc.sync.dma_start(
                    out=out_d[t * P : t * P + rows, :], in_=hrow[:rows, : g.h2]
                )

            def tab1(b):
                lo = b * bsz1
                hi = min(g.n_nodes, lo + bsz1)
                return x_d[lo:hi, :]

            def tab2(b):
                lo = b * bsz2
                hi = min(npair2, lo + bsz2)
                return hf_d[lo:hi, :]

            def self1(t0, kk):
                return xo_d[t0 * P : (t0 + kk) * P, :].rearrange(
                    "(t i) e -> i t e", i=P
                )

            def self2(t0, kk):
                return hb_d[t0 * P : (t0 + kk) * P, :].rearrange(
                    "(t i b) c -> i t (b c)", i=64, b=2
                )

            layer(gi1_d, dl1_d, nb1, lay1, tab1, self1, elem1, g.in_dim, False,
                  False, epi_l1)

            tc.strict_bb_all_engine_barrier()
            if os.environ.get("GCN_NOCC", "0") == "1":  # debug: skip collective
                nc.sync.dma_start(
                    out=hf_d[: shard_pad // 2, :],
                    in_=hb_d[:, :].rearrange("(a b) c -> a (b c)", b=2),
                )
            else:
                # bf16 AllGather was observed to wedge the exec unit at
                # >=512KB per rank; it is pure data movement, so ship the
                # same bytes as f32.
                cc_in = hb_d.ap() if not g.mm_bf16 else hb_d.ap().bitcast(f32)
                cc_out = hf_d.ap() if not g.mm_bf16 else hf_d.ap().bitcast(f32)
                nc.gpsimd.collective_compute(
                    "AllGather",
                    mybir.AluOpType.bypass,
                    replica_groups=[list(range(g.n_cores))],
                    ins=[cc_in.opt()],
                    outs=[cc_out.opt()],
                )
            tc.strict_bb_all_engine_barrier()

            layer(gi2_d, dl2_d, nb2, lay2, tab2, self2, elem2, g.h2, True,
                  True, epi_l2)

    nc.compile()
    return nc


_PROGRAM_CACHE: dict = {}
LAST_RESULTS = None  # BassKernelResults of the most recent kernel() call


def _layout_key(layout):
    def lk(lay):
        return (
            lay["nb"],
            lay["n_mm"],
            tuple(lay["calls"]),
            tuple(tuple(s) for s in lay["sched"]),
        )

    return (lk(layout["l1"]), lk(layout["l2"]))


def _get_program(g: Geom, layout):
    key = (g, _layout_key(layout))
    if key not in _PROGRAM_CACHE:
        _PROGRAM_CACHE[key] = build_program(g, layout)
    return _PROGRAM_CACHE[key]


def host_consts(g: Geom):
    import ml_dtypes

    tdt = ml_dtypes.bfloat16 if g.mm_bf16 else np.float32
    iotam = np.tile(np.arange(2 * P, dtype=np.float32), (P, 1)).astype(tdt)
    sself = np.zeros((P // 2, 2 * P), dtype=np.float32)
    for i in range(P // 2):
        sself[i, 2 * i] = 1.0
        sself[i, P + 2 * i + 1] = 1.0
    ident = np.eye(P, dtype=np.float32)
    return dict(iotam=iotam, sself=sself.astype(tdt), identm=ident.astype(tdt))


def run(x, edge_index, W1, b1, W2, b2, g: Geom, trace: bool = False):
    global LAST_RESULTS
    import ml_dtypes
    from concourse.bass_utils import run_bass_kernel_spmd

    per_core, layout, dinv = preprocess(np.asarray(edge_index), g)
    nc = _get_program(g, layout)

    tdt = ml_dtypes.bfloat16 if g.mm_bf16 else np.float32
    consts = host_consts(g)
    xs = np.asarray(x) * dinv[:, None]  # fold dinv[src] into the table
    x_t = np.ascontiguousarray(xs).astype(tdt)
    w1_t = np.asarray(W1).astype(tdt)
    w2_t = np.asarray(W2).astype(tdt)
    b1_t = np.asarray(b1).astype(np.float32)
    b2_t = np.asarray(b2).astype(np.float32)[None, :]

    xo_pad = np.zeros((g.shard_pad, g.in_dim), dtype=tdt)
    in_maps = []
    for i, pc in enumerate(per_core):
        lo = i * g.shard
        xo = xo_pad.copy()
        xo[: g.shard] = x_t[lo : lo + g.shard]
        in_maps.append(
            dict(
                x=x_t, xown=xo, gidx1=pc["gidx1"], gidx2=pc["gidx2"],
                dl1=pc["dl1"], dl2=pc["dl2"], ddst=pc["ddst"], sdeg=pc["sdeg"],
                w1=w1_t, w2=w2_t, b1=b1_t, b2row=b2_t, **consts,
            )
        )

    core_ids = list(range(g.n_cores))
    if trace:
        try:
            res = run_bass_kernel_spmd(
                nc, in_maps, core_ids=core_ids, trace=True, trace_cores=[0]
            )
        except Exception as e:  # fall back to an untraced run
            print(f"[kernel] traced run failed ({type(e).__name__}: {e}); retrying untraced")
            res = run_bass_kernel_spmd(nc, in_maps, core_ids=core_ids)
    else:
        res = run_bass_kernel_spmd(nc, in_maps, core_ids=core_ids)
    LAST_RESULTS = res
    out = np.concatenate([r["out"] for r in res.results], axis=0)
    return out[: g.n_nodes]


_FULL = Geom(
    n_nodes=100000,
    n_cores=8,
    in_dim=128,
    h1=128,
    h2=64,
    gcols1=int(os.environ.get("GCN_GCOLS1", "7")),
    gcols2=int(os.environ.get("GCN_GCOLS2", "7")),
    selfk=int(os.environ.get("GCN_SELFK", "14")),
    mm_bf16=os.environ.get("GCN_F32", "0") != "1",
)


def kernel(x, edge_index, W1, b1, W2, b2):
    trace = os.environ.get("GCN_TRACE", "0") == "1"
    return run(x, edge_index, W1, b1, W2, b2, _FULL, trace=trace)
